# revision 27
# baseline (speedup 1.0000x reference)
"""Distributed causal self-attention kernel for 8 TRN2 NeuronCores (Bass/Tile).

Self-contained: kernel(**inputs) takes the FULL unsharded inputs
(x [2,4096,768], W_kqv [12,768,192], W_proj [768,768], b_proj [768]),
shards them across 8 cores (batch x head-group), runs one SPMD NEFF via
bass_utils.run_bass_kernel_spmd, and reassembles the full [2,4096,768] output.

v3 (vs the v2 RS-pipelined kernel):
 - No collective: each core writes its full y^T partial (row-sharded output
   projection) to DRAM; the host sums the 4 head-group partials per batch
   during unsharding.  Removes the ReduceScatter chain, the CC-core peer
   waits, and the DMA bounce tail.
 - PE mode-switch batching: S-matmul row-tile pairs and full-array ctx
   matmuls are issued in G=2 groups ([4xS][4xctx]) instead of per-j
   alternation; kqv/y-projection matmuls are scheduled inside the full-mode
   windows.  Measured ~190ns/MM vs ~250ns/MM for per-j alternation.
 - Causal diag masks via gpsimd affine_select on the exp output (idle
   engine) instead of DVE multiplies.
 - Softmax reciprocal via the custom-DVE reciprocal_approx_fast (~5x) and
   denominator broadcast via gpsimd partition_broadcast instead of fp32r
   matmuls (removes PE stalls between the AB and C loops).
 - kC/qC swap halves built by DVE copies instead of duplicated projection
   columns (weff 704 -> 576 cols, -48 matmuls).
"""

import sys

for p in ("/opt/trn_rl_repo", "/root/.axon_site/_ro/trn_rl_repo"):
    if p not in sys.path:
        sys.path.insert(0, p)


import ml_dtypes
import numpy as np

import concourse.bass as bass
import concourse.bass_isa as bass_isa
import concourse.mybir as mybir
import concourse.tile as tile
from concourse.masks import make_identity

F32 = mybir.dt.float32
BF16 = mybir.dt.bfloat16
EXPF = mybir.ActivationFunctionType.Exp


class Cfg:
    def __init__(self, N=4096, D=768, H=12, B=2, NCORES=8):
        self.N, self.D, self.H, self.B, self.NCORES = N, D, H, B, NCORES
        self.HD = D // H          # 64
        self.KB = 128             # k block
        self.NKB = N // self.KB   # 32 k blocks
        self.QT = N // NCORES     # q tile (512)
        self.NT = N // self.QT    # 8 q tiles
        self.R = self.QT // self.KB  # diag masks per q tile (4)
        self.KC = D // 128        # contraction chunks (6)
        assert self.HD == 64 and self.QT == 512


def build(tc: tile.TileContext, out_y: bass.AP, ins: dict, cfg: Cfg):
    nc = tc.nc
    ctx_lp = nc.allow_low_precision(reason="bf16 matmul pipeline")
    ctx_lp.__enter__()
    N, D, QT, KB, R, KC, NT = cfg.N, cfg.D, cfg.QT, cfg.KB, cfg.R, cfg.KC, cfg.NT
    HD = cfg.HD
    VW = HD + 1  # 65: v block + ones column (denominator rides the ctx matmul)
    scale = 1.0 / np.sqrt(HD)
    xT, weff, wp01, wp2 = ins["xT"], ins["weff"], ins["wp01"], ins["wp2"]

    persist = tc.alloc_tile_pool(name="persist", bufs=1)

    # --- persistent tiles (allocations only; setup instructions deferred
    # so the chunk-0 kqv projection starts the PE as early as possible) ---
    F32R = mybir.dt.float32r
    ident = persist.tile([128, 64], BF16)
    selst = persist.tile([65, 128], F32)
    sel2 = persist.tile([33, 128], BF16)
    sel3 = persist.tile([65, 64], BF16)

    # kqv weights: 5 col tiles of 128/64: 0:[kA|kB] 1:[qA|qB] 2:[kC|qC]
    # 3:[vA|vB] 4:[vC]
    WEC = 9 * HD  # 576
    we_sb = []
    for kc in range(KC):
        w = persist.tile([128, WEC], BF16, name=f"we{kc}")
        nc.sync.dma_start(w[:], weff[128 * kc:128 * (kc + 1), :])
        we_sb.append(w)
    wp01_sb = persist.tile([128, D], BF16, name="wp01")
    nc.sync.dma_start(wp01_sb[:], wp01[:])
    wp2_sb = persist.tile([65, D], BF16, name="wp2")
    nc.sync.dma_start(wp2_sb[:], wp2[:])

    # persistent activation tensors
    kq_ab = persist.tile([128, 2 * N], BF16)   # p0:64 kA|qA, p64:128 kB|qB
    kq_c = persist.tile([128, N], BF16)        # p0:64 kC, p64:128 qC
    kq_cs = persist.tile([128, N], BF16)       # swap: p0:64 qC, p64:128 kC
    vones3 = [persist.tile([128, cfg.NKB, VW], BF16, name=f"vones{hi}")
              for hi in range(3)]
    cn01_t = [persist.tile([128, QT], BF16, name=f"cn01_{i}") for i in range(2)]
    cn2_t = [persist.tile([65, QT], BF16, name=f"cn2_{i}") for i in range(2)]

    # diag masks: mask_d[p, c] = 1.0 if c >= KB*d + p else 0
    masks = [persist.tile([128, QT], BF16, name=f"mask{d}") for d in range(R)]

    def emit_setup_v():
        # identity replicated in both partition halves for the v transposes;
        # vones ones-columns at [:, j, 64] (den rides the ctx matmul)
        make_identity(nc, ident[0:64, :])
        make_identity(nc, ident[64:128, :])
        for v in vones3:
            nc.vector.memset(v[:], 1.0)
        for d in range(R):
            nc.vector.memset(masks[d][:], 1.0)
            nc.gpsimd.affine_select(
                out=masks[d][:], in_=masks[d][:],
                compare_op=mybir.AluOpType.is_ge, fill=0.0,
                base=-KB * d, pattern=[[1, QT]], channel_multiplier=-1)

    def emit_setup_norm():
        # sel matmuls broadcast the per-head reciprocal rows into partition
        # halves (den rows at partitions 0/32/64; matmul base-partition rule)
        nc.vector.memset(selst[:], 0.0)
        nc.vector.memset(selst[0:1, 0:64], 1.0)
        nc.vector.memset(selst[32:33, 64:128], 1.0)
        nc.vector.tensor_copy(sel2[:], selst[0:33, :])
        nc.vector.memset(selst[:], 0.0)
        nc.vector.memset(selst[64:65, 0:64], 1.0)
        nc.vector.tensor_copy(sel3[:], selst[:, 0:64])
        for t_ in cn2_t:
            nc.vector.memset(t_[:], 1.0)  # row 64 = ones (bias row of wp2)

    xt_p = tc.alloc_tile_pool(name="xt", bufs=13)
    vstage_p = tc.alloc_tile_pool(name="vstage", bufs=2)
    ex_p = tc.alloc_tile_pool(name="exp_sb", bufs=3)
    small_p = tc.alloc_tile_pool(name="small", bufs=4)
    ysb_p = tc.alloc_tile_pool(name="ysb", bufs=2)
    cnst_p = tc.alloc_tile_pool(name="cnst", bufs=4)
    # PSUM budget (8 banks): s_ps 2x2 + ctx 2x1 + misc 2x1 = 8
    s_psp = tc.alloc_tile_pool(name="s_ps", bufs=2, space="PSUM")
    ctx_psp = tc.alloc_tile_pool(name="ctx_ps", bufs=2, space="PSUM")
    misc_psp = tc.alloc_tile_pool(name="misc_ps", bufs=2, space="PSUM")

    # ---------------- kqv projection, one 512-token chunk ----------------
    def emit_A_fillers(t):
        """DMA the chunk's x tiles now; return thunks (to be run inside
        full-array-mode windows) for the 5 weight-tile matmul groups."""
        xts = []
        for kc in range(KC):
            xt_sb = xt_p.tile([128, QT], BF16, name="xt_sb")
            nc.sync.dma_start(xt_sb[:], xT[t, 128 * kc:128 * (kc + 1), :])
            xts.append(xt_sb)

        def group(mt):
            mw = 128 if mt < 4 else 64
            ps = misc_psp.tile([128, QT], F32, name="kqv_ps", tag="misc", bufs=2)
            for kc in range(KC):
                nc.tensor.matmul(
                    ps[0:mw, :],
                    we_sb[kc][:, 128 * mt:128 * mt + mw],
                    xts[kc][:],
                    start=(kc == 0), stop=(kc == KC - 1))
            if mt == 0:
                nc.vector.tensor_copy(kq_ab[:, QT * t:QT * (t + 1)], ps[:])
            elif mt == 1:
                nc.vector.tensor_copy(
                    kq_ab[:, N + QT * t:N + QT * (t + 1)], ps[:])
            elif mt == 2:
                nc.vector.tensor_copy(kq_c[:, QT * t:QT * (t + 1)], ps[:])
                # swap halves for the C-loop even/odd pairing
                nc.vector.tensor_copy(
                    kq_cs[0:64, QT * t:QT * (t + 1)], ps[64:128, :])
                nc.vector.tensor_copy(
                    kq_cs[64:128, QT * t:QT * (t + 1)], ps[0:64, :])
            else:
                vst = vstage_p.tile([128, QT], BF16, name="vst")
                nc.vector.tensor_copy(vst[0:mw, :], ps[0:mw, :])
                for (hi, po) in ([(0, 0), (1, 64)] if mt == 3 else [(2, 0)]):
                    vtp = misc_psp.tile([128, 4, 64], BF16, name="vtp",
                                        tag="misc", bufs=2)
                    for ch in range(4):
                        nc.tensor.transpose(
                            vtp[:, ch],
                            vst[po:po + 64, 128 * ch:128 * (ch + 1)],
                            ident[po:po + 64, :])
                    nc.vector.tensor_copy(
                        vones3[hi][:, 4 * t:4 * t + 4, 0:64], vtp[:, :, :])

        return [(lambda mt=mt: group(mt)) for mt in range(5)]

    def emit_A(t):
        for th in emit_A_fillers(t):
            th()

    # ---------------- flash attention (S^T layout) ----------------
    def norm_stage(ctx_ps, dall, hi, csb, po):
        nc.vector.tensor_copy(dall[32 * hi:32 * hi + 1, :], ctx_ps[64:65, :])
        nc.vector.tensor_copy(csb[po:po + 64, :], ctx_ps[0:64, :])

    def norm_recip(dall):
        r_f = small_p.tile([65, QT], F32, name="r_f")
        nc.vector.reciprocal_approx_fast(r_f[:], dall[:])
        r_r = small_p.tile([65, QT], BF16, name="r_r")
        nc.vector.tensor_copy(r_r[:], r_f[:])
        return r_r

    def norm_mul01(r_r, csb01, dst):
        bc = misc_psp.tile([128, QT], F32, name="bc_ps", tag="misc", bufs=2)
        nc.tensor.matmul(
            bc[:], sel2[:], r_r[0:33, :], start=True, stop=True)
        nc.vector.tensor_mul(dst, csb01[:], bc[:])

    def norm_mul2(r_r, csb2, dst):
        bc = misc_psp.tile([64, QT], F32, name="bc_ps", tag="misc", bufs=2)
        nc.tensor.matmul(
            bc[:], sel3[:], r_r[:], start=True, stop=True)
        nc.vector.tensor_mul(dst, csb2[:], bc[:])

    def mask_ex(ex_slice, d):
        # zero ex[p, c] where c < KB*d + p  (upper triangle of diag block d)
        nc.vector.tensor_mul(ex_slice, ex_slice, masks[d][:])

    # heads A,B advance together: row-tiled S pair fills the 128x128 array.
    # G=2 batching: [S-pair(j) S-pair(j+1)] then [flush(j) flush(j+1)] so the
    # PE sees 4 row-mode matmuls then a full-mode window (ctx + fillers).
    def emit_AB(qt, fillers=()):
        fillers = list(fillers)
        njb = R * (qt + 1)
        ctxA = ctx_psp.tile([VW, QT], F32, name="ctxA", tag="ctx", bufs=2)
        ctxB = ctx_psp.tile([VW, QT], F32, name="ctxB", tag="ctx", bufs=2)
        qA = kq_ab[0:64, N + QT * qt:N + QT * (qt + 1)]
        qB = kq_ab[64:128, N + QT * qt:N + QT * (qt + 1)]

        def spair(j):
            s_ps = s_psp.tile([128, 2 * QT], F32, name="s_ps", tag="s", bufs=2)
            nc.tensor.matmul(
                s_ps[:, 0:QT], kq_ab[0:64, KB * j:KB * (j + 1)], qA,
                start=True, stop=True, tile_position=(0, 0))
            nc.tensor.matmul(
                s_ps[:, QT:2 * QT], kq_ab[64:128, KB * j:KB * (j + 1)], qB,
                start=True, stop=True, tile_position=(64, 0))
            return s_ps

        def flush(j, s_ps):
            ex = ex_p.tile([128, 2 * QT], BF16, name="ex")
            nc.scalar.activation(ex[:], s_ps[:], EXPF, scale=scale)
            d = j - R * qt
            if d >= 0:
                for h in range(2):
                    mask_ex(ex[:, QT * h:QT * (h + 1)], d)
            nc.tensor.matmul(
                ctxA[:], vones3[0][:, j], ex[:, 0:QT],
                start=(j == 0), stop=(j == njb - 1))
            nc.tensor.matmul(
                ctxB[:], vones3[1][:, j], ex[:, QT:2 * QT],
                start=(j == 0), stop=(j == njb - 1))

        pend = []
        nf0 = len(fillers)
        for jg in range(0, njb, 2):
            s0 = spair(jg)
            s1 = spair(jg + 1)
            for (j, sp) in pend:
                flush(j, sp)
            # fillers run in the full-mode window right after the ctx mms
            if fillers and jg >= (nf0 - len(fillers)) * njb // (nf0 + 1):
                fillers.pop(0)()
            pend = [(jg, s0), (jg + 1, s1)]
        for (j, sp) in pend:
            flush(j, sp)
        while fillers:
            fillers.pop(0)()
        return ctxA, ctxB

    # head C: pair even/odd k-blocks via the two kq_c layouts.
    # G=2: two s_ps tiles (4 j blocks) then two flushes.
    def emit_C(qt, fillers=()):
        fillers = list(fillers)
        njb = R * (qt + 1)
        ctxC = ctx_psp.tile([VW, QT], F32, name="ctxC", tag="ctx", bufs=2)
        qslice = slice(QT * qt, QT * (qt + 1))

        def spairC(jg):
            # even j at tile (0,0): kC and qC both from partitions 0:64
            # odd j at tile (64,0): kC and qC both from partitions 64:128
            s_ps = s_psp.tile([128, 2 * QT], F32, name="s_psC", tag="s", bufs=2)
            nc.tensor.matmul(
                s_ps[:, 0:QT],
                kq_c[0:64, KB * jg:KB * (jg + 1)],
                kq_cs[0:64, qslice],
                start=True, stop=True, tile_position=(0, 0))
            nc.tensor.matmul(
                s_ps[:, QT:2 * QT],
                kq_cs[64:128, KB * (jg + 1):KB * (jg + 2)],
                kq_c[64:128, qslice],
                start=True, stop=True, tile_position=(64, 0))
            return s_ps

        def flushC(j0, s_ps, last):
            ex = ex_p.tile([128, 2 * QT], BF16, name="exC")
            nc.scalar.activation(ex[:], s_ps[:], EXPF, scale=scale)
            for i in (0, 1):
                d = j0 + i - R * qt
                if d >= 0:
                    mask_ex(ex[:, QT * i:QT * (i + 1)], d)
            nc.tensor.matmul(
                ctxC[:], vones3[2][:, j0], ex[:, 0:QT],
                start=(j0 == 0), stop=False)
            nc.tensor.matmul(
                ctxC[:], vones3[2][:, j0 + 1], ex[:, QT:2 * QT],
                start=False, stop=last)

        pend = []
        for jg in range(0, njb, 4):
            s0 = spairC(jg)
            s1 = spairC(jg + 2) if jg + 2 < njb else None
            for (j0, sp) in pend:
                flushC(j0, sp, j0 + 1 == njb - 1)
            if fillers:
                fillers.pop(0)()
            pend = [(jg, s0)]
            if s1 is not None:
                pend.append((jg + 2, s1))
        for (j0, sp) in pend:
            flushC(j0, sp, j0 + 1 == njb - 1)
        while fillers:
            fillers.pop(0)()
        return ctxC

    # ---------------- per-q-tile output projection (no collective) --------
    # y^T partial over local heads: [128 outdims, 512 toks] per out-chunk,
    # written straight to DRAM; host sums the 4 head-group partials.
    def emit_y_fillers(qt):
        cn01 = cn01_t[qt % 2]
        cn2 = cn2_t[qt % 2]
        ysb = ysb_p.tile([128, 6, QT], BF16, name="ysb")

        def chunk(c):
            yt = misc_psp.tile([128, QT], F32, name="y_ps", tag="misc", bufs=2)
            nc.tensor.matmul(
                yt[:], wp01_sb[:, 128 * c:128 * (c + 1)], cn01[:],
                start=True, stop=False)
            nc.tensor.matmul(
                yt[:], wp2_sb[:, 128 * c:128 * (c + 1)], cn2[:],
                start=False, stop=True)
            nc.vector.tensor_copy(ysb[:, c, :], yt[:])

        return ysb, [(lambda c=c: chunk(c)) for c in range(6)]

    def finish_y(qt, ysb):
        nc.sync.dma_start(out_y[qt], ysb[:])

    # ---------------- pipelined schedule ----------------
    # chunk 0: tiles 0 (kAB), 1 (qAB), 3 (vAB) gate the first AB loop;
    # head-C tiles 2/4 ride the filler queue into AB(0).  Setup DVE work is
    # interleaved after the k/q projections so S(0) starts early.
    a0 = emit_A_fillers(0)
    a0[0]()
    a0[1]()
    emit_setup_v()
    a0[3]()
    emit_setup_norm()
    pend_norm = None   # (r, csb01, csb2, qt) awaiting its muls
    pend_y = None      # qt whose y-chunks go into the next C loop
    for qt in range(NT):
        afill = emit_A_fillers(qt + 1) if qt + 1 < NT else []
        if qt == 0:
            afill = [a0[2], a0[4]] + afill
        if pend_norm is not None:
            # consume the previous tile's normalization as an early filler
            # inside this AB loop so its y-projection can overlap emit_C
            r_p, c01_p, c2_p, qp = pend_norm

            def norm_thunk(r_p=r_p, c01_p=c01_p, c2_p=c2_p, qp=qp):
                norm_mul01(r_p, c01_p, cn01_t[qp % 2][:, :])
                norm_mul2(r_p, c2_p, cn2_t[qp % 2][0:64, :])

            afill = [norm_thunk] + afill
            pend_y = qp
            pend_norm = None
        ctxA, ctxB = emit_AB(qt, afill)
        dall = small_p.tile([65, QT], F32, name="dall")
        nc.vector.memset(dall[:], 1.0)  # unused lanes stay finite for recip
        csb01 = cnst_p.tile([128, QT], BF16, name="csb01")
        csb2 = cnst_p.tile([64, QT], BF16, name="csb2")
        norm_stage(ctxA, dall, 0, csb01, 0)
        norm_stage(ctxB, dall, 1, csb01, 64)
        if pend_y is not None:
            ysb_p_, ythunks = emit_y_fillers(pend_y)
            ctxC = emit_C(qt, ythunks)
            finish_y(pend_y, ysb_p_)
            pend_y = None
        else:
            ctxC = emit_C(qt)
        norm_stage(ctxC, dall, 2, csb2, 0)
        r_f = norm_recip(dall)
        pend_norm = (r_f, csb01, csb2, qt)
    r_p, c01_p, c2_p, qp = pend_norm
    norm_mul01(r_p, c01_p, cn01_t[qp % 2][:, :])
    norm_mul2(r_p, c2_p, cn2_t[qp % 2][0:64, :])
    ysb_l, ythunks = emit_y_fillers(qp)
    for th in ythunks:
        th()
    finish_y(qp, ysb_l)

    misc_psp.release()
    ctx_psp.release()
    s_psp.release()
    cnst_p.release()
    ysb_p.release()
    small_p.release()
    ex_p.release()
    vstage_p.release()
    xt_p.release()
    persist.release()
    ctx_lp.__exit__(None, None, None)


def shard_inputs(x, W_kqv, W_proj, b_proj, cfg: Cfg):
    """Full inputs -> list of 8 per-core input dicts (numpy, host layout)."""
    HD = cfg.HD
    in_maps = []
    x = np.asarray(x, np.float32)
    W_kqv = np.asarray(W_kqv, np.float32)
    wpT = np.ascontiguousarray(np.asarray(W_proj, np.float32).T)  # [in, out]
    b_proj = np.asarray(b_proj, np.float32)
    for c in range(cfg.NCORES):
        b = c // 4
        g = c % 4
        hs = [3 * g, 3 * g + 1, 3 * g + 2]
        k = [W_kqv[h][:, 0:HD] for h in hs]
        q = [W_kqv[h][:, HD:2 * HD] for h in hs]
        v = [W_kqv[h][:, 2 * HD:3 * HD] for h in hs]
        # col tiles: [kA|kB] [qA|qB] [kC|qC] [vA|vB] [vC]
        weff = np.concatenate(
            [k[0], k[1], q[0], q[1], k[2], q[2],
             v[0], v[1], v[2]], axis=1).astype(np.float32)
        wp01 = wpT[192 * g:192 * g + 128, :]
        wp2 = np.zeros((65, cfg.D), np.float32)
        wp2[0:64] = wpT[192 * g + 128:192 * g + 192, :]
        if g == 0:
            wp2[64] = b_proj  # bias folded in exactly once per batch group
        # chunk-blocked x^T: [NT, D, QT] so each [128, 512] tile DMA is one
        # contiguous 128KB transfer instead of 128 strided 1KB descriptors
        xTb = np.ascontiguousarray(
            x[b].T.reshape(cfg.D, cfg.NT, cfg.QT).transpose(1, 0, 2))
        in_maps.append({
            "xT": xTb.astype(ml_dtypes.bfloat16),
            "weff": np.ascontiguousarray(weff).astype(ml_dtypes.bfloat16),
            "wp01": np.ascontiguousarray(wp01).astype(ml_dtypes.bfloat16),
            "wp2": np.ascontiguousarray(wp2).astype(ml_dtypes.bfloat16),
        })
    return in_maps


def assemble_output(outs, cfg: Cfg):
    """Per-core y^T partials [NT, 128, 6, QT] -> full [B, N, D] (host sum)."""
    y = np.zeros((cfg.B, cfg.N, cfg.D), np.float32)
    for b in range(cfg.B):
        ybT = np.zeros((cfg.D, cfg.N), np.float32)
        for g in range(4):
            o = np.asarray(outs[4 * b + g], np.float32)  # [qt, od_i, oc, tok]
            ybT += o.transpose(2, 1, 0, 3).reshape(cfg.D, cfg.N)
        y[b] = ybT.T
    return y


_NC_CACHE = {}


def _build_nc(cfg):
    from concourse import bacc

    nc = bacc.Bacc(
        "TRN2", target_bir_lowering=False, debug=False,
        num_devices=cfg.NCORES)
    ins = {
        "xT": nc.dram_tensor("xT", [cfg.NT, cfg.D, cfg.QT], BF16,
                             kind="ExternalInput").ap(),
        "weff": nc.dram_tensor("weff", [cfg.D, 9 * cfg.HD], BF16,
                               kind="ExternalInput").ap(),
        "wp01": nc.dram_tensor("wp01", [128, cfg.D], BF16,
                               kind="ExternalInput").ap(),
        "wp2": nc.dram_tensor("wp2", [65, cfg.D], BF16,
                              kind="ExternalInput").ap(),
    }
    out = nc.dram_tensor("y", [cfg.NT, 128, 6, cfg.QT], BF16,
                         kind="ExternalOutput").ap()
    with tile.TileContext(nc) as tc:
        build(tc, out, ins, cfg)
    nc.compile()
    return nc


def _get_nc(cfg):
    if "nc" not in _NC_CACHE:
        _NC_CACHE["nc"] = _build_nc(cfg)
    return _NC_CACHE["nc"]


def run_sharded(inputs, trace=False):
    import concourse.bass_utils as bass_utils

    cfg = Cfg(N=4096)
    in_maps = shard_inputs(
        inputs["x"], inputs["W_kqv"], inputs["W_proj"], inputs["b_proj"], cfg)
    nc = _get_nc(cfg)
    res = bass_utils.run_bass_kernel_spmd(
        nc, in_maps, core_ids=list(range(cfg.NCORES)), trace=trace)
    outs = [res.results[c]["y"] for c in range(cfg.NCORES)]
    return assemble_output(outs, cfg), res


def kernel(**inputs):
    y, _ = run_sharded(inputs, trace=False)
    return y


# revision 28
# speedup vs baseline: 1.0067x; 1.0067x over previous
"""Distributed causal self-attention kernel for 8 TRN2 NeuronCores (Bass/Tile).

Self-contained: kernel(**inputs) takes the FULL unsharded inputs
(x [2,4096,768], W_kqv [12,768,192], W_proj [768,768], b_proj [768]),
shards them across 8 cores (batch x head-group), runs one SPMD NEFF via
bass_utils.run_bass_kernel_spmd, and reassembles the full [2,4096,768] output.

v3 (vs the v2 RS-pipelined kernel):
 - No collective: each core writes its full y^T partial (row-sharded output
   projection) to DRAM; the host sums the 4 head-group partials per batch
   during unsharding.  Removes the ReduceScatter chain, the CC-core peer
   waits, and the DMA bounce tail.
 - PE mode-switch batching: S-matmul row-tile pairs and full-array ctx
   matmuls are issued in G=2 groups ([4xS][4xctx]) instead of per-j
   alternation; kqv/y-projection matmuls are scheduled inside the full-mode
   windows.  Measured ~190ns/MM vs ~250ns/MM for per-j alternation.
 - Causal diag masks via gpsimd affine_select on the exp output (idle
   engine) instead of DVE multiplies.
 - Softmax reciprocal via the custom-DVE reciprocal_approx_fast (~5x) and
   denominator broadcast via gpsimd partition_broadcast instead of fp32r
   matmuls (removes PE stalls between the AB and C loops).
 - kC/qC swap halves built by DVE copies instead of duplicated projection
   columns (weff 704 -> 576 cols, -48 matmuls).
"""

import sys

for p in ("/opt/trn_rl_repo", "/root/.axon_site/_ro/trn_rl_repo"):
    if p not in sys.path:
        sys.path.insert(0, p)


import ml_dtypes
import numpy as np

import concourse.bass as bass
import concourse.bass_isa as bass_isa
import concourse.mybir as mybir
import concourse.tile as tile
from concourse.masks import make_identity

F32 = mybir.dt.float32
BF16 = mybir.dt.bfloat16
EXPF = mybir.ActivationFunctionType.Exp


class Cfg:
    def __init__(self, N=4096, D=768, H=12, B=2, NCORES=8):
        self.N, self.D, self.H, self.B, self.NCORES = N, D, H, B, NCORES
        self.HD = D // H          # 64
        self.KB = 128             # k block
        self.NKB = N // self.KB   # 32 k blocks
        self.QT = N // NCORES     # q tile (512)
        self.NT = N // self.QT    # 8 q tiles
        self.R = self.QT // self.KB  # diag masks per q tile (4)
        self.KC = D // 128        # contraction chunks (6)
        assert self.HD == 64 and self.QT == 512


def build(tc: tile.TileContext, out_y: bass.AP, ins: dict, cfg: Cfg):
    nc = tc.nc
    ctx_lp = nc.allow_low_precision(reason="bf16 matmul pipeline")
    ctx_lp.__enter__()
    N, D, QT, KB, R, KC, NT = cfg.N, cfg.D, cfg.QT, cfg.KB, cfg.R, cfg.KC, cfg.NT
    HD = cfg.HD
    VW = HD + 1  # 65: v block + ones column (denominator rides the ctx matmul)
    scale = 1.0 / np.sqrt(HD)
    xT, weff, wp01, wp2 = ins["xT"], ins["weff"], ins["wp01"], ins["wp2"]

    persist = tc.alloc_tile_pool(name="persist", bufs=1)

    # --- persistent tiles (allocations only; setup instructions deferred
    # so the chunk-0 kqv projection starts the PE as early as possible) ---
    F32R = mybir.dt.float32r
    ident = persist.tile([128, 64], BF16)
    selst = persist.tile([65, 128], F32)
    sel2 = persist.tile([33, 128], BF16)
    sel3 = persist.tile([65, 64], BF16)

    # kqv weights: 5 col tiles of 128/64: 0:[kA|kB] 1:[qA|qB] 2:[kC|qC]
    # 3:[vA|vB] 4:[vC]
    WEC = 9 * HD  # 576
    we_sb = []
    for kc in range(KC):
        w = persist.tile([128, WEC], BF16, name=f"we{kc}")
        nc.sync.dma_start(w[:], weff[128 * kc:128 * (kc + 1), :])
        we_sb.append(w)
    wp01_sb = persist.tile([128, D], BF16, name="wp01")
    nc.sync.dma_start(wp01_sb[:], wp01[:])
    wp2_sb = persist.tile([65, D], BF16, name="wp2")
    nc.sync.dma_start(wp2_sb[:], wp2[:])

    # persistent activation tensors
    kq_ab = persist.tile([128, 2 * N], BF16)   # p0:64 kA|qA, p64:128 kB|qB
    kq_c = persist.tile([128, N], BF16)        # p0:64 kC, p64:128 qC
    kq_cs = persist.tile([128, N], BF16)       # swap: p0:64 qC, p64:128 kC
    vones3 = [persist.tile([128, cfg.NKB, VW], BF16, name=f"vones{hi}")
              for hi in range(3)]
    cn01_t = [persist.tile([128, QT], BF16, name=f"cn01_{i}") for i in range(2)]
    cn2_t = [persist.tile([65, QT], BF16, name=f"cn2_{i}") for i in range(2)]

    # diag masks: mask_d[p, c] = 1.0 if c >= KB*d + p else 0
    masks = [persist.tile([128, QT], BF16, name=f"mask{d}") for d in range(R)]

    def emit_setup_v():
        # identity replicated in both partition halves for the v transposes;
        # vones ones-columns at [:, j, 64] (den rides the ctx matmul)
        make_identity(nc, ident[0:64, :])
        make_identity(nc, ident[64:128, :])
        for v in vones3:
            nc.vector.memset(v[:], 1.0)
        for d in range(R):
            nc.vector.memset(masks[d][:], 1.0)
            nc.gpsimd.affine_select(
                out=masks[d][:], in_=masks[d][:],
                compare_op=mybir.AluOpType.is_ge, fill=0.0,
                base=-KB * d, pattern=[[1, QT]], channel_multiplier=-1)

    def emit_setup_norm():
        # sel matmuls broadcast the per-head reciprocal rows into partition
        # halves (den rows at partitions 0/32/64; matmul base-partition rule)
        nc.vector.memset(selst[:], 0.0)
        nc.vector.memset(selst[0:1, 0:64], 1.0)
        nc.vector.memset(selst[32:33, 64:128], 1.0)
        nc.vector.tensor_copy(sel2[:], selst[0:33, :])
        nc.vector.memset(selst[:], 0.0)
        nc.vector.memset(selst[64:65, 0:64], 1.0)
        nc.vector.tensor_copy(sel3[:], selst[:, 0:64])
        for t_ in cn2_t:
            nc.vector.memset(t_[:], 1.0)  # row 64 = ones (bias row of wp2)

    xt_p = tc.alloc_tile_pool(name="xt", bufs=13)
    vstage_p = tc.alloc_tile_pool(name="vstage", bufs=2)
    ex_p = tc.alloc_tile_pool(name="exp_sb", bufs=3)
    small_p = tc.alloc_tile_pool(name="small", bufs=4)
    ysb_p = tc.alloc_tile_pool(name="ysb", bufs=2)
    cnst_p = tc.alloc_tile_pool(name="cnst", bufs=4)
    # PSUM budget (8 banks): s_ps 2x2 + ctx 2x1 + misc 2x1 = 8
    s_psp = tc.alloc_tile_pool(name="s_ps", bufs=2, space="PSUM")
    ctx_psp = tc.alloc_tile_pool(name="ctx_ps", bufs=2, space="PSUM")
    misc_psp = tc.alloc_tile_pool(name="misc_ps", bufs=2, space="PSUM")

    # ---------------- kqv projection, one 512-token chunk ----------------
    def emit_A_fillers(t):
        """DMA the chunk's x tiles now; return thunks (to be run inside
        full-array-mode windows) for the 5 weight-tile matmul groups."""
        xts = []
        for kc in range(KC):
            xt_sb = xt_p.tile([128, QT], BF16, name="xt_sb")
            nc.sync.dma_start(xt_sb[:], xT[t, 128 * kc:128 * (kc + 1), :])
            xts.append(xt_sb)

        def group(mt):
            mw = 128 if mt < 4 else 64
            ps = misc_psp.tile([128, QT], F32, name="kqv_ps", tag="misc", bufs=2)
            for kc in range(KC):
                nc.tensor.matmul(
                    ps[0:mw, :],
                    we_sb[kc][:, 128 * mt:128 * mt + mw],
                    xts[kc][:],
                    start=(kc == 0), stop=(kc == KC - 1))
            if mt == 0:
                nc.vector.tensor_copy(kq_ab[:, QT * t:QT * (t + 1)], ps[:])
            elif mt == 1:
                nc.vector.tensor_copy(
                    kq_ab[:, N + QT * t:N + QT * (t + 1)], ps[:])
            elif mt == 2:
                nc.vector.tensor_copy(kq_c[:, QT * t:QT * (t + 1)], ps[:])
                # swap halves for the C-loop even/odd pairing
                nc.vector.tensor_copy(
                    kq_cs[0:64, QT * t:QT * (t + 1)], ps[64:128, :])
                nc.vector.tensor_copy(
                    kq_cs[64:128, QT * t:QT * (t + 1)], ps[0:64, :])
            else:
                vst = vstage_p.tile([128, QT], BF16, name="vst")
                nc.vector.tensor_copy(vst[0:mw, :], ps[0:mw, :])
                for (hi, po) in ([(0, 0), (1, 64)] if mt == 3 else [(2, 0)]):
                    vtp = misc_psp.tile([128, 4, 64], BF16, name="vtp",
                                        tag="misc", bufs=2)
                    for ch in range(4):
                        nc.tensor.transpose(
                            vtp[:, ch],
                            vst[po:po + 64, 128 * ch:128 * (ch + 1)],
                            ident[po:po + 64, :])
                    nc.vector.tensor_copy(
                        vones3[hi][:, 4 * t:4 * t + 4, 0:64], vtp[:, :, :])

        return [(lambda mt=mt: group(mt)) for mt in range(5)]

    def emit_A(t):
        for th in emit_A_fillers(t):
            th()

    # ---------------- flash attention (S^T layout) ----------------
    def norm_stage(ctx_ps, dall, hi, csb, po):
        nc.vector.tensor_copy(dall[32 * hi:32 * hi + 1, :], ctx_ps[64:65, :])
        nc.vector.tensor_copy(csb[po:po + 64, :], ctx_ps[0:64, :])

    def norm_recip(dall):
        r_f = small_p.tile([65, QT], F32, name="r_f")
        nc.vector.reciprocal_approx_fast(r_f[:], dall[:])
        r_r = small_p.tile([65, QT], BF16, name="r_r")
        nc.vector.tensor_copy(r_r[:], r_f[:])
        return r_r

    def norm_mul01(r_r, csb01, dst):
        bc = misc_psp.tile([128, QT], F32, name="bc_ps", tag="misc", bufs=2)
        nc.tensor.matmul(
            bc[:], sel2[:], r_r[0:33, :], start=True, stop=True)
        nc.vector.tensor_mul(dst, csb01[:], bc[:])

    def norm_mul2(r_r, csb2, dst):
        bc = misc_psp.tile([64, QT], F32, name="bc_ps", tag="misc", bufs=2)
        nc.tensor.matmul(
            bc[:], sel3[:], r_r[:], start=True, stop=True)
        nc.vector.tensor_mul(dst, csb2[:], bc[:])

    def mask_ex(ex_slice, d):
        # zero ex[p, c] where c < KB*d + p  (upper triangle of diag block d)
        nc.vector.tensor_mul(ex_slice, ex_slice, masks[d][:])

    # heads A,B advance together: row-tiled S pair fills the 128x128 array.
    # G=2 batching: [S-pair(j) S-pair(j+1)] then [flush(j) flush(j+1)] so the
    # PE sees 4 row-mode matmuls then a full-mode window (ctx + fillers).
    def emit_AB(qt, fillers=()):
        fillers = list(fillers)
        njb = R * (qt + 1)
        ctxA = ctx_psp.tile([VW, QT], F32, name="ctxA", tag="ctx", bufs=2)
        ctxB = ctx_psp.tile([VW, QT], F32, name="ctxB", tag="ctx", bufs=2)
        qA = kq_ab[0:64, N + QT * qt:N + QT * (qt + 1)]
        qB = kq_ab[64:128, N + QT * qt:N + QT * (qt + 1)]

        def spair(j):
            s_ps = s_psp.tile([128, 2 * QT], F32, name="s_ps", tag="s", bufs=2)
            nc.tensor.matmul(
                s_ps[:, 0:QT], kq_ab[0:64, KB * j:KB * (j + 1)], qA,
                start=True, stop=True, tile_position=(0, 0))
            nc.tensor.matmul(
                s_ps[:, QT:2 * QT], kq_ab[64:128, KB * j:KB * (j + 1)], qB,
                start=True, stop=True, tile_position=(64, 0))
            return s_ps

        def flush(j, s_ps):
            ex = ex_p.tile([128, 2 * QT], BF16, name="ex")
            nc.scalar.activation(ex[:], s_ps[:], EXPF, scale=scale)
            d = j - R * qt
            if d >= 0:
                for h in range(2):
                    mask_ex(ex[:, QT * h:QT * (h + 1)], d)
            nc.tensor.matmul(
                ctxA[:], vones3[0][:, j], ex[:, 0:QT],
                start=(j == 0), stop=(j == njb - 1))
            nc.tensor.matmul(
                ctxB[:], vones3[1][:, j], ex[:, QT:2 * QT],
                start=(j == 0), stop=(j == njb - 1))

        pend = []
        nf0 = len(fillers)
        for jg in range(0, njb, 2):
            s0 = spair(jg)
            s1 = spair(jg + 1)
            for (j, sp) in pend:
                flush(j, sp)
            # fillers run in the full-mode window right after the ctx mms
            if fillers and jg >= (nf0 - len(fillers)) * njb // (nf0 + 1):
                fillers.pop(0)()
            pend = [(jg, s0), (jg + 1, s1)]
        for (j, sp) in pend:
            flush(j, sp)
        while fillers:
            fillers.pop(0)()
        return ctxA, ctxB

    # head C: pair even/odd k-blocks via the two kq_c layouts.
    # G=2: two s_ps tiles (4 j blocks) then two flushes.
    def emit_C(qt, fillers=()):
        fillers = list(fillers)
        njb = R * (qt + 1)
        ctxC = ctx_psp.tile([VW, QT], F32, name="ctxC", tag="ctx", bufs=2)
        qslice = slice(QT * qt, QT * (qt + 1))

        def spairC(jg):
            # even j at tile (0,0): kC and qC both from partitions 0:64
            # odd j at tile (64,0): kC and qC both from partitions 64:128
            s_ps = s_psp.tile([128, 2 * QT], F32, name="s_psC", tag="s", bufs=2)
            nc.tensor.matmul(
                s_ps[:, 0:QT],
                kq_c[0:64, KB * jg:KB * (jg + 1)],
                kq_cs[0:64, qslice],
                start=True, stop=True, tile_position=(0, 0))
            nc.tensor.matmul(
                s_ps[:, QT:2 * QT],
                kq_cs[64:128, KB * (jg + 1):KB * (jg + 2)],
                kq_c[64:128, qslice],
                start=True, stop=True, tile_position=(64, 0))
            return s_ps

        def flushC(j0, s_ps, last):
            ex = ex_p.tile([128, 2 * QT], BF16, name="exC")
            nc.scalar.activation(ex[:], s_ps[:], EXPF, scale=scale)
            for i in (0, 1):
                d = j0 + i - R * qt
                if d >= 0:
                    mask_ex(ex[:, QT * i:QT * (i + 1)], d)
            nc.tensor.matmul(
                ctxC[:], vones3[2][:, j0], ex[:, 0:QT],
                start=(j0 == 0), stop=False)
            nc.tensor.matmul(
                ctxC[:], vones3[2][:, j0 + 1], ex[:, QT:2 * QT],
                start=False, stop=last)

        pend = []
        for jg in range(0, njb, 4):
            s0 = spairC(jg)
            s1 = spairC(jg + 2) if jg + 2 < njb else None
            for (j0, sp) in pend:
                flushC(j0, sp, j0 + 1 == njb - 1)
            if fillers:
                fillers.pop(0)()
            pend = [(jg, s0)]
            if s1 is not None:
                pend.append((jg + 2, s1))
        for (j0, sp) in pend:
            flushC(j0, sp, j0 + 1 == njb - 1)
        while fillers:
            fillers.pop(0)()
        return ctxC

    # ---------------- per-q-tile output projection (no collective) --------
    # y^T partial over local heads: [128 outdims, 512 toks] per out-chunk,
    # written straight to DRAM; host sums the 4 head-group partials.
    def emit_y_fillers(qt):
        cn01 = cn01_t[qt % 2]
        cn2 = cn2_t[qt % 2]
        ysb = ysb_p.tile([128, 6, QT], BF16, name="ysb")

        def chunk(c):
            yt = misc_psp.tile([128, QT], F32, name="y_ps", tag="misc", bufs=2)
            nc.tensor.matmul(
                yt[:], wp01_sb[:, 128 * c:128 * (c + 1)], cn01[:],
                start=True, stop=False)
            nc.tensor.matmul(
                yt[:], wp2_sb[:, 128 * c:128 * (c + 1)], cn2[:],
                start=False, stop=True)
            nc.vector.tensor_copy(ysb[:, c, :], yt[:])

        return ysb, [(lambda c=c: chunk(c)) for c in range(6)]

    def finish_y(qt, ysb):
        nc.sync.dma_start(out_y[qt], ysb[:])

    # ---------------- pipelined schedule ----------------
    # chunk 0: tiles 0 (kAB), 1 (qAB), 3 (vAB) gate the first AB loop;
    # head-C tiles 2/4 ride the filler queue into AB(0).  Setup DVE work is
    # interleaved after the k/q projections so S(0) starts early.
    a0 = emit_A_fillers(0)
    a0[0]()
    a0[1]()
    emit_setup_v()
    a0[3]()
    emit_setup_norm()
    pend_norm = None   # (r, csb01, csb2, qt) awaiting its muls
    pend_y = None      # qt whose y-chunks go into the next C loop
    for qt in range(NT):
        afill = emit_A_fillers(qt + 1) if qt + 1 < NT else []
        if qt == 0:
            afill = [a0[2], a0[4]] + afill
        ctxA, ctxB = emit_AB(qt, afill)
        if pend_norm is not None:
            r_p, c01_p, c2_p, qp = pend_norm
            norm_mul01(r_p, c01_p, cn01_t[qp % 2][:, :])
            norm_mul2(r_p, c2_p, cn2_t[qp % 2][0:64, :])
            pend_y = qp
            pend_norm = None
        dall = small_p.tile([65, QT], F32, name="dall")
        nc.vector.memset(dall[:], 1.0)  # unused lanes stay finite for recip
        csb01 = cnst_p.tile([128, QT], BF16, name="csb01")
        csb2 = cnst_p.tile([64, QT], BF16, name="csb2")
        norm_stage(ctxA, dall, 0, csb01, 0)
        norm_stage(ctxB, dall, 1, csb01, 64)
        if pend_y is not None:
            ysb_p_, ythunks = emit_y_fillers(pend_y)
            ctxC = emit_C(qt, ythunks)
            finish_y(pend_y, ysb_p_)
            pend_y = None
        else:
            ctxC = emit_C(qt)
        norm_stage(ctxC, dall, 2, csb2, 0)
        r_f = norm_recip(dall)
        pend_norm = (r_f, csb01, csb2, qt)
    r_p, c01_p, c2_p, qp = pend_norm
    norm_mul01(r_p, c01_p, cn01_t[qp % 2][:, :])
    norm_mul2(r_p, c2_p, cn2_t[qp % 2][0:64, :])
    ysb_l, ythunks = emit_y_fillers(qp)
    for th in ythunks:
        th()
    finish_y(qp, ysb_l)

    misc_psp.release()
    ctx_psp.release()
    s_psp.release()
    cnst_p.release()
    ysb_p.release()
    small_p.release()
    ex_p.release()
    vstage_p.release()
    xt_p.release()
    persist.release()
    ctx_lp.__exit__(None, None, None)


def shard_inputs(x, W_kqv, W_proj, b_proj, cfg: Cfg):
    """Full inputs -> list of 8 per-core input dicts (numpy, host layout)."""
    HD = cfg.HD
    in_maps = []
    x = np.asarray(x, np.float32)
    W_kqv = np.asarray(W_kqv, np.float32)
    wpT = np.ascontiguousarray(np.asarray(W_proj, np.float32).T)  # [in, out]
    b_proj = np.asarray(b_proj, np.float32)
    for c in range(cfg.NCORES):
        b = c // 4
        g = c % 4
        hs = [3 * g, 3 * g + 1, 3 * g + 2]
        k = [W_kqv[h][:, 0:HD] for h in hs]
        q = [W_kqv[h][:, HD:2 * HD] for h in hs]
        v = [W_kqv[h][:, 2 * HD:3 * HD] for h in hs]
        # col tiles: [kA|kB] [qA|qB] [kC|qC] [vA|vB] [vC]
        weff = np.concatenate(
            [k[0], k[1], q[0], q[1], k[2], q[2],
             v[0], v[1], v[2]], axis=1).astype(np.float32)
        wp01 = wpT[192 * g:192 * g + 128, :]
        wp2 = np.zeros((65, cfg.D), np.float32)
        wp2[0:64] = wpT[192 * g + 128:192 * g + 192, :]
        if g == 0:
            wp2[64] = b_proj  # bias folded in exactly once per batch group
        # chunk-blocked x^T: [NT, D, QT] so each [128, 512] tile DMA is one
        # contiguous 128KB transfer instead of 128 strided 1KB descriptors
        xTb = np.ascontiguousarray(
            x[b].T.reshape(cfg.D, cfg.NT, cfg.QT).transpose(1, 0, 2))
        in_maps.append({
            "xT": xTb.astype(ml_dtypes.bfloat16),
            "weff": np.ascontiguousarray(weff).astype(ml_dtypes.bfloat16),
            "wp01": np.ascontiguousarray(wp01).astype(ml_dtypes.bfloat16),
            "wp2": np.ascontiguousarray(wp2).astype(ml_dtypes.bfloat16),
        })
    return in_maps


def assemble_output(outs, cfg: Cfg):
    """Per-core y^T partials [NT, 128, 6, QT] -> full [B, N, D] (host sum)."""
    y = np.zeros((cfg.B, cfg.N, cfg.D), np.float32)
    for b in range(cfg.B):
        ybT = np.zeros((cfg.D, cfg.N), np.float32)
        for g in range(4):
            o = np.asarray(outs[4 * b + g], np.float32)  # [qt, od_i, oc, tok]
            ybT += o.transpose(2, 1, 0, 3).reshape(cfg.D, cfg.N)
        y[b] = ybT.T
    return y


_NC_CACHE = {}


def _build_nc(cfg):
    from concourse import bacc

    nc = bacc.Bacc(
        "TRN2", target_bir_lowering=False, debug=False,
        num_devices=cfg.NCORES)
    ins = {
        "xT": nc.dram_tensor("xT", [cfg.NT, cfg.D, cfg.QT], BF16,
                             kind="ExternalInput").ap(),
        "weff": nc.dram_tensor("weff", [cfg.D, 9 * cfg.HD], BF16,
                               kind="ExternalInput").ap(),
        "wp01": nc.dram_tensor("wp01", [128, cfg.D], BF16,
                               kind="ExternalInput").ap(),
        "wp2": nc.dram_tensor("wp2", [65, cfg.D], BF16,
                              kind="ExternalInput").ap(),
    }
    out = nc.dram_tensor("y", [cfg.NT, 128, 6, cfg.QT], BF16,
                         kind="ExternalOutput").ap()
    with tile.TileContext(nc) as tc:
        build(tc, out, ins, cfg)
    nc.compile()
    return nc


def _get_nc(cfg):
    if "nc" not in _NC_CACHE:
        _NC_CACHE["nc"] = _build_nc(cfg)
    return _NC_CACHE["nc"]


def run_sharded(inputs, trace=False):
    import concourse.bass_utils as bass_utils

    cfg = Cfg(N=4096)
    in_maps = shard_inputs(
        inputs["x"], inputs["W_kqv"], inputs["W_proj"], inputs["b_proj"], cfg)
    nc = _get_nc(cfg)
    res = bass_utils.run_bass_kernel_spmd(
        nc, in_maps, core_ids=list(range(cfg.NCORES)), trace=trace)
    outs = [res.results[c]["y"] for c in range(cfg.NCORES)]
    return assemble_output(outs, cfg), res


def kernel(**inputs):
    y, _ = run_sharded(inputs, trace=False)
    return y
